# revision 13
# baseline (speedup 1.0000x reference)
"""Window-gather variant: edges sorted by src, grouped by 512-row z windows.
Src rows are rebuilt on-chip: PE one-hot matmuls (sel = is_equal(iota, srcrel)
generated on DVE) against sequentially-streamed z window tiles, costing zero
Pool-queue time. Only the dst side uses indirect DMA gathers (the ~1.45us/instr
SWDGE serialization wall). Slot counts per group are maxed across cores so one
SPMD program serves all 8. z stored bf16; bf16 PE path. Host unpermutes via
orig-id map."""

import numpy as np
import ml_dtypes

import concourse.bass as bass
import concourse.mybir as mybir
import concourse.tile as tile
from concourse import bacc
from concourse.bass import IndirectOffsetOnAxis
from concourse.bass_utils import run_bass_kernel_spmd
from concourse.masks import make_identity
from contextlib import ExitStack

N, D, H = 100000, 128, 128
E_TOTAL = 2000000
NCORES = 8
P = 128
GW = 4          # windows per group
GROUP_ROWS = GW * P   # 512 z-rows per group
NG = -(-N // GROUP_ROWS)  # 196 groups
NPAD = NG * GROUP_ROWS    # z padded to 100352 rows
E_CORE = E_TOTAL // NCORES
SLM = 16        # max slots per group (assert at pack time)

F32 = mybir.dt.float32
F16 = mybir.dt.float16
BF16 = mybir.dt.bfloat16
I32 = mybir.dt.int32
RELU = mybir.ActivationFunctionType.Relu
IDENT = mybir.ActivationFunctionType.Identity
EQ = mybir.AluOpType.is_equal


def build_program(counts, tot):
    nc = bacc.Bacc("TRN2", target_bir_lowering=False, debug=False,
                   enable_asserts=False, num_devices=NCORES)
    z_d = nc.dram_tensor("z", [NPAD, D], BF16, kind="ExternalInput").ap()
    idx_d = nc.dram_tensor("idx", [P, tot], I32, kind="ExternalInput").ap()
    srcrel_d = nc.dram_tensor("srcrel", [tot * P], F16, kind="ExternalInput").ap()
    w1_d = nc.dram_tensor("w1", [D, H], F32, kind="ExternalInput").ap()
    b1_d = nc.dram_tensor("b1", [H], F32, kind="ExternalInput").ap()
    w2_d = nc.dram_tensor("w2", [H, 1], F32, kind="ExternalInput").ap()
    b2_d = nc.dram_tensor("b2", [1], F32, kind="ExternalInput").ap()
    out_d = nc.dram_tensor("out", [tot * P], F32, kind="ExternalOutput").ap()

    with tile.TileContext(nc) as tc, ExitStack() as ctx:
        const = ctx.enter_context(tc.tile_pool(name="const", bufs=1))
        wpool = ctx.enter_context(tc.tile_pool(name="win", bufs=3))
        stp = ctx.enter_context(tc.tile_pool(name="srcT", bufs=2))
        selp = ctx.enter_context(tc.tile_pool(name="sel", bufs=2))
        zdp = ctx.enter_context(tc.tile_pool(name="zd", bufs=3))
        efp = ctx.enter_context(tc.tile_pool(name="ef", bufs=2))
        work = ctx.enter_context(tc.tile_pool(name="work", bufs=3))
        stage = ctx.enter_context(tc.tile_pool(name="stage", bufs=2))
        ps_om = ctx.enter_context(tc.tile_pool(name="ps_om", bufs=2, space="PSUM"))
        ps_s = ctx.enter_context(tc.tile_pool(name="ps_s", bufs=2, space="PSUM"))
        ps_t = ctx.enter_context(tc.tile_pool(name="ps_t", bufs=2, space="PSUM"))
        ps_h = ctx.enter_context(tc.tile_pool(name="ps_h", bufs=1, space="PSUM"))
        ps_o = ctx.enter_context(tc.tile_pool(name="ps_o", bufs=1, space="PSUM"))

        idx_sb = const.tile([P, tot], I32)
        nc.sync.dma_start(out=idx_sb[:], in_=idx_d[:, :])
        w1f = const.tile([P, H], F32)
        nc.sync.dma_start(out=w1f[:], in_=w1_d[:, :])
        b1_sb = const.tile([P, 1], F32)
        nc.sync.dma_start(out=b1_sb[:], in_=b1_d[:, None])
        w2f = const.tile([P, 1], F32)
        nc.sync.dma_start(out=w2f[:], in_=w2_d[:, :])
        b2_sb = const.tile([1, 1], F32)
        nc.sync.dma_start(out=b2_sb[:1], in_=b2_d[:, None])
        w1_sb = const.tile([P, H], BF16)
        nc.vector.tensor_copy(out=w1_sb[:], in_=w1f[:])
        w2_sb = const.tile([P, 1], BF16)
        nc.vector.tensor_copy(out=w2_sb[:], in_=w2f[:])
        ident = const.tile([P, P], BF16)
        make_identity(nc, ident[:])
        ones_sb = const.tile([1, P], F16)
        nc.gpsimd.memset(ones_sb[:1], 1.0)
        iota4 = const.tile([P, GW], F32)
        for j in range(GW):
            nc.gpsimd.iota(out=iota4[:, j : j + 1], pattern=[[0, 1]], base=P * j,
                           channel_multiplier=1,
                           allow_small_or_imprecise_dtypes=True)

        def gather(dst_ap, col0):
            nc.gpsimd.indirect_dma_start(
                out=dst_ap, out_offset=None, in_=z_d[:, :],
                in_offset=IndirectOffsetOnAxis(ap=idx_sb[:, col0 : col0 + 1],
                                               axis=0),
            )

        slotbase = 0
        for g in range(NG):
            cnt = counts[g]
            if cnt == 0:
                continue
            win = wpool.tile([P, GW * D], BF16, tag="win")
            for j in range(GW):
                r0 = GROUP_ROWS * g + P * j
                nc.sync.dma_start(out=win[:, j * D : (j + 1) * D],
                                  in_=z_d[r0 : r0 + P, :])
            srcT = stp.tile([1, SLM * P], F16, tag="srcT")
            nc.sync.dma_start(
                out=srcT[:1, : cnt * P],
                in_=srcrel_d[slotbase * P : (slotbase + cnt) * P][None, :])
            o_stage = stage.tile([1, SLM * P], F32, tag="ost")
            for b0 in range(0, cnt, 4):
                nb = min(4, cnt - b0)
                EB = nb * P
                som = ps_om.tile([P, 512], F32)
                nc.tensor.matmul(out=som[:, :EB], lhsT=ones_sb[:1, :],
                                 rhs=srcT[:1, b0 * P : (b0 + nb) * P],
                                 start=True, stop=True)
                sel = selp.tile([P, GW * 512], BF16, tag="sel")
                for j in range(GW):
                    nc.vector.tensor_tensor(
                        out=sel[:, j * 512 : j * 512 + EB],
                        in0=iota4[:, j : j + 1].to_broadcast([P, EB]),
                        in1=som[:, :EB], op=EQ)
                zt = zdp.tile([P, 512], BF16, tag="zd")
                for sl in range(nb):
                    gather(zt[:, sl * D : (sl + 1) * D], slotbase + b0 + sl)
                ztT_ps = ps_t.tile([P, 512], BF16)
                for c in range(nb):
                    nc.tensor.transpose(
                        out=ztT_ps[:, c * P : (c + 1) * P],
                        in_=zt[:, c * D : (c + 1) * D], identity=ident[:])
                ztT = work.tile([P, 512], BF16, tag="ztT")
                nc.vector.tensor_copy(out=ztT[:, :EB], in_=ztT_ps[:, :EB])
                spsT = ps_s.tile([P, 512], F32)
                for j in range(GW):
                    nc.tensor.matmul(
                        out=spsT[:, :EB], lhsT=win[:, j * D : (j + 1) * D],
                        rhs=sel[:, j * 512 : j * 512 + EB],
                        start=(j == 0), stop=(j == GW - 1))
                efT = efp.tile([P, 512], BF16, tag="ef")
                nc.vector.tensor_mul(out=efT[:, :EB], in0=spsT[:, :EB],
                                     in1=ztT[:, :EB])
                h_ps = ps_h.tile([P, 512], F32)
                nc.tensor.matmul(out=h_ps[:, :EB], lhsT=w1_sb[:],
                                 rhs=efT[:, :EB], start=True, stop=True)
                h_sb = work.tile([P, 512], BF16, tag="h")
                nc.scalar.activation(out=h_sb[:, :EB], in_=h_ps[:, :EB],
                                     func=RELU, bias=b1_sb[:, :1], scale=1.0)
                o_ps = ps_o.tile([1, 512], F32)
                nc.tensor.matmul(out=o_ps[:1, :EB], lhsT=w2_sb[:],
                                 rhs=h_sb[:, :EB], start=True, stop=True)
                nc.scalar.activation(
                    out=o_stage[:1, b0 * P : b0 * P + EB], in_=o_ps[:1, :EB],
                    func=IDENT, bias=b2_sb[:1, :1], scale=1.0)
            nc.sync.dma_start(
                out=out_d[slotbase * P : (slotbase + cnt) * P][None, :],
                in_=o_stage[:1, : cnt * P])
            slotbase += cnt

    nc.compile()
    return nc


def pack_all(edge_label_index, e_core=E_CORE):
    """Balanced shard: edges sorted by src globally, each group's edges dealt
    round-robin across cores (evens per-group slot counts). Returns per-core
    (idx [P,tot] i32, ORIG [P,tot] i64 GLOBAL edge ids, srcrel [tot*P] f16)
    + uniform per-group slot counts + tot."""
    src_f = np.asarray(edge_label_index[0], dtype=np.int64)
    dst_f = np.asarray(edge_label_index[1], dtype=np.int64)
    order = np.argsort(src_f, kind="stable")
    ss, dd = src_f[order], dst_f[order]
    bounds = np.searchsorted(ss, np.arange(1, NG + 1) * GROUP_ROWS)
    starts = np.concatenate([[0], bounds[:-1]])
    percore = [[] for _ in range(NCORES)]
    counts = np.zeros(NG, np.int64)
    for g in range(NG):
        st, en = int(starts[g]), int(bounds[g])
        mmax = 0
        for c in range(NCORES):
            seg = np.arange(st + c, en, NCORES)
            percore[c].append(seg)
            mmax = max(mmax, len(seg))
        counts[g] = -(-mmax // P)
    assert counts.max() <= SLM, counts.max()
    tot = int(counts.sum())
    packed = []
    for c in range(NCORES):
        idx = np.zeros((P, tot), np.int32)
        ORIG = np.full((P, tot), -1, np.int64)
        srcrel = np.full(tot * P, -1.0, np.float16)
        sb = 0
        for g in range(NG):
            cnt = int(counts[g])
            if cnt == 0:
                continue
            seg = percore[c][g]
            k = len(seg)
            srcrel[sb * P : sb * P + k] = (
                ss[seg] - GROUP_ROWS * g).astype(np.float16)
            lanes = np.arange(k) % P
            slots = sb + np.arange(k) // P
            idx[lanes, slots] = dd[seg]
            ORIG[lanes, slots] = order[seg]
            sb += cnt
        packed.append((idx, ORIG, srcrel))
    return packed, counts, tot


_NC_CACHE = {}


def run(inputs, trace=False, **kw):
    z = np.asarray(inputs["z"], dtype=np.float32)
    zp = np.zeros((NPAD, D), np.float32)
    zp[:N] = z
    zb = np.ascontiguousarray(zp.astype(ml_dtypes.bfloat16))
    w1 = np.ascontiguousarray(np.asarray(inputs["W1"], dtype=np.float32))
    b1v = np.ascontiguousarray(np.asarray(inputs["b1"], dtype=np.float32))
    w2 = np.ascontiguousarray(np.asarray(inputs["W2"], dtype=np.float32))
    b2v = np.ascontiguousarray(np.asarray(inputs["b2"], dtype=np.float32))
    packed, counts, tot = pack_all(inputs["edge_label_index"])
    key = (tuple(counts), tot)
    if key not in _NC_CACHE:
        _NC_CACHE[key] = build_program(counts, tot)
    res = run_bass_kernel_spmd(
        _NC_CACHE[key],
        [{"z": zb, "idx": idx, "srcrel": srcrel, "w1": w1, "b1": b1v,
          "w2": w2, "b2": b2v}
         for idx, _, srcrel in packed],
        list(range(NCORES)), trace=trace, **kw)
    full = np.zeros(E_TOTAL, np.float32)
    for c in range(NCORES):
        dev = res.results[c]["out"]
        orig_flat = packed[c][1].T.ravel()
        valid = orig_flat >= 0
        full[orig_flat[valid]] = dev[valid]
    return full, res


def kernel(z, edge_label_index, W1, b1, W2, b2):
    out, _ = run({"z": z, "edge_label_index": edge_label_index,
                  "W1": W1, "b1": b1, "W2": W2, "b2": b2})
    return out
